# revision 7
# baseline (speedup 1.0000x reference)
import sys

if "/opt/trn_rl_repo" not in sys.path:
    sys.path.insert(0, "/opt/trn_rl_repo")

from contextlib import ExitStack

import numpy as np

import concourse.mybir as mybir
from concourse import bacc
from concourse.bass_utils import run_bass_kernel_spmd
from concourse.masks import make_identity
from concourse.tile import TileContext

F32 = mybir.dt.float32

B, T, C, H, D = 8, 512, 1024, 16, 64
MAX_POS = 512
TOPK = 32
P = 128
OT = C // P  # 8 channel tiles
TT = T // P  # 4 token tiles
N_CORES = 8

# Matmul input dtypes (float32 = exact-ish 4cyc/row; float32r = fast 1cyc/row)
QK_MM_DT = F32   # scores path: must be accurate (top-k selection)
V_MM_DT = F32
PV_MM_DT = F32
OUT_MM_DT = F32


def _mm(ap, dt):
    return ap.bitcast(dt) if dt != F32 else ap


def build_program():
    nc = bacc.Bacc(None, target_bir_lowering=False)

    xT_d = nc.declare_dram_parameter("xT", [P, OT, T], F32, isOutput=False)
    wq_d = nc.declare_dram_parameter("wq", [P, OT, C], F32, isOutput=False)
    wk_d = nc.declare_dram_parameter("wk", [P, OT, C], F32, isOutput=False)
    wv_d = nc.declare_dram_parameter("wv", [P, OT, C], F32, isOutput=False)
    wo_d = nc.declare_dram_parameter("wo", [P, OT, C], F32, isOutput=False)
    bqp_d = nc.declare_dram_parameter("bqp", [P, OT], F32, isOutput=False)
    bkp_d = nc.declare_dram_parameter("bkp", [P, OT], F32, isOutput=False)
    bvb_d = nc.declare_dram_parameter("bvb", [P, C], F32, isOutput=False)
    bob_d = nc.declare_dram_parameter("bob", [P, C], F32, isOutput=False)
    gates_d = nc.declare_dram_parameter("gates", [P, H], F32, isOutput=False)
    posb_d = nc.declare_dram_parameter("posb", [H, TT, P, T], F32, isOutput=False)
    out_d = nc.declare_dram_parameter("out", [T, C], F32, isOutput=True)

    Exp = mybir.ActivationFunctionType.Exp
    add = mybir.AluOpType.add
    mult = mybir.AluOpType.mult
    is_ge = mybir.AluOpType.is_ge

    with TileContext(nc) as tc, ExitStack() as ctx:
        const = ctx.enter_context(tc.tile_pool(name="const", bufs=1))
        wpool = ctx.enter_context(tc.tile_pool(name="wpool", bufs=2))
        xpool = ctx.enter_context(tc.tile_pool(name="xpool", bufs=1))
        proj = ctx.enter_context(tc.tile_pool(name="proj", bufs=1))
        spool = ctx.enter_context(tc.tile_pool(name="spool", bufs=2))
        ppool = ctx.enter_context(tc.tile_pool(name="ppool", bufs=8))
        small = ctx.enter_context(tc.tile_pool(name="small", bufs=4))
        biasp = ctx.enter_context(tc.tile_pool(name="biasp", bufs=3))
        outp = ctx.enter_context(tc.tile_pool(name="outp", bufs=2))
        psA = ctx.enter_context(tc.tile_pool(name="psA", bufs=2, space="PSUM"))
        psS = ctx.enter_context(tc.tile_pool(name="psS", bufs=2, space="PSUM"))
        psT = ctx.enter_context(tc.tile_pool(name="psT", bufs=2, space="PSUM"))
        psO = ctx.enter_context(tc.tile_pool(name="psO", bufs=2, space="PSUM"))

        ident = const.tile([P, P], F32)
        make_identity(nc, ident)
        gates_sb = const.tile([P, H], F32)
        nc.sync.dma_start(gates_sb[:], gates_d[:])
        bqp_sb = const.tile([P, OT], F32)
        nc.sync.dma_start(bqp_sb[:], bqp_d[:])
        bkp_sb = const.tile([P, OT], F32)
        nc.sync.dma_start(bkp_sb[:], bkp_d[:])
        bvb_sb = const.tile([P, C], F32)
        nc.sync.dma_start(bvb_sb[:], bvb_d[:])
        bob_sb = const.tile([P, C], F32)
        nc.sync.dma_start(bob_sb[:], bob_d[:])

        xT_sb = xpool.tile([P, OT, T], F32)
        nc.sync.dma_start(xT_sb[:], xT_d[:])

        wq_sb = wpool.tile([P, OT, C], F32, tag="w")
        nc.sync.dma_start(wq_sb[:], wq_d[:])
        wk_sb = wpool.tile([P, OT, C], F32, tag="w")
        nc.sync.dma_start(wk_sb[:], wk_d[:])

        # ---- Q/K projections: out layout [channel, token] (transposed) ----
        QT_sb = proj.tile([P, OT, T], F32, tag="qt")
        KT_sb = proj.tile([P, OT, T], F32, tag="kt")
        for w_sb, bias_sb, dst in ((wq_sb, bqp_sb, QT_sb), (wk_sb, bkp_sb, KT_sb)):
            for ot in range(OT):
                ps = psA.tile([P, T], F32, tag="psA")
                for kt in range(OT):
                    nc.tensor.matmul(
                        ps[:],
                        lhsT=_mm(w_sb[:, kt, ot * P:(ot + 1) * P], QK_MM_DT),
                        rhs=_mm(xT_sb[:, kt, :], QK_MM_DT),
                        start=(kt == 0),
                        stop=(kt == OT - 1),
                    )
                nc.vector.tensor_scalar_add(dst[:, ot, :], ps[:], bias_sb[:, ot:ot + 1])

        # ---- V projection: natural layout [token, channel] ----
        wv_sb = wpool.tile([P, OT, C], F32, tag="w")
        nc.sync.dma_start(wv_sb[:], wv_d[:])
        V_sb = proj.tile([P, TT, C], F32, tag="v")
        for tt in range(TT):
            for oh in range(2):
                ps = psA.tile([P, T], F32, tag="psA")
                for kt in range(OT):
                    nc.tensor.matmul(
                        ps[:],
                        lhsT=_mm(xT_sb[:, kt, tt * P:(tt + 1) * P], V_MM_DT),
                        rhs=_mm(wv_sb[:, kt, oh * 512:(oh + 1) * 512], V_MM_DT),
                        start=(kt == 0),
                        stop=(kt == OT - 1),
                    )
                nc.vector.tensor_tensor(
                    V_sb[:, tt, oh * 512:(oh + 1) * 512], ps[:],
                    bvb_sb[:, oh * 512:(oh + 1) * 512], op=add,
                )

        wo_sb = wpool.tile([P, OT, C], F32, tag="w")
        nc.sync.dma_start(wo_sb[:], wo_d[:])

        # ---- attention, head pair g = (2g, 2g+1) ----
        AO_sb = proj.tile([P, OT, T], F32, tag="ao")
        for g in range(OT):
            ao_ps = psO.tile([P, T], F32, tag="psO")
            for hh in range(2):
                h = 2 * g + hh
                prow = 64 * hh
                p_tiles = []
                for it in range(TT):
                    s_ps = psS.tile([P, T], F32, tag="psS")
                    nc.tensor.matmul(
                        s_ps[:],
                        lhsT=_mm(QT_sb[prow:prow + 64, g, it * P:(it + 1) * P], QK_MM_DT),
                        rhs=_mm(KT_sb[prow:prow + 64, g, :], QK_MM_DT),
                        start=True,
                        stop=True,
                    )
                    pb_sb = biasp.tile([P, T], F32, tag="pb")
                    nc.sync.dma_start(pb_sb[:], posb_d[h, it])
                    S_sb = spool.tile([P, T], F32, tag="S")
                    nc.vector.tensor_tensor(S_sb[:], s_ps[:], pb_sb[:], op=add)

                    # top-32 threshold: 4 rounds of max8 / match_replace
                    m0 = small.tile([P, 8], F32, tag="m0")
                    nc.vector.max(out=m0[:], in_=S_sb[:])
                    sc0 = spool.tile([P, T], F32, tag="sc0")
                    nc.vector.match_replace(out=sc0[:], in_to_replace=m0[:],
                                            in_values=S_sb[:], imm_value=-1e30)
                    m1 = small.tile([P, 8], F32, tag="m1")
                    nc.vector.max(out=m1[:], in_=sc0[:])
                    sc1 = spool.tile([P, T], F32, tag="sc1")
                    nc.vector.match_replace(out=sc1[:], in_to_replace=m1[:],
                                            in_values=sc0[:], imm_value=-1e30)
                    m2 = small.tile([P, 8], F32, tag="m2")
                    nc.vector.max(out=m2[:], in_=sc1[:])
                    sc2 = spool.tile([P, T], F32, tag="sc0")
                    nc.vector.match_replace(out=sc2[:], in_to_replace=m2[:],
                                            in_values=sc1[:], imm_value=-1e30)
                    m3 = small.tile([P, 8], F32, tag="m3")
                    nc.vector.max(out=m3[:], in_=sc2[:])
                    sc3 = spool.tile([P, T], F32, tag="sc1")
                    nc.vector.match_replace(out=sc3[:], in_to_replace=m3[:],
                                            in_values=sc2[:], imm_value=-1e30)

                    negmx = small.tile([P, 1], F32, tag="negmx")
                    nc.vector.tensor_scalar_mul(negmx[:], m0[:, 0:1], -1.0)
                    # E = exp(S - mx); E4 = exp(S4 - mx): identical except at the
                    # top-32 positions (where E4 is 0), so E - E4 is the masked
                    # softmax numerator and sumE - sumE4 its normalizer.
                    E = spool.tile([P, T], F32, tag="E")
                    sumE = small.tile([P, 1], F32, tag="sumE")
                    nc.scalar.activation(E[:], S_sb[:], Exp, bias=negmx[:], scale=1.0,
                                         accum_out=sumE[:])
                    E4 = spool.tile([P, T], F32, tag="E4")
                    sumE4 = small.tile([P, 1], F32, tag="sumE4")
                    nc.scalar.activation(E4[:], sc3[:], Exp, bias=negmx[:], scale=1.0,
                                         accum_out=sumE4[:])

                    sum_ = small.tile([P, 1], F32, tag="sum")
                    nc.vector.tensor_sub(sum_[:], sumE[:], sumE4[:])
                    inv = small.tile([P, 1], F32, tag="inv")
                    nc.vector.reciprocal(inv[:], sum_[:])
                    scl = small.tile([P, 1], F32, tag="scl")
                    nc.vector.tensor_tensor(scl[:], inv[:], gates_sb[:, h:h + 1], op=mult)

                    p_t = ppool.tile([P, T], F32, tag="P")
                    nc.gpsimd.tensor_sub(p_t[:], E[:], E4[:])
                    nc.gpsimd.tensor_scalar_mul(p_t[:], p_t[:], scl[:])
                    p_tiles.append(p_t)

                # transpose P and accumulate attn_out^T = V^T-free matmul
                for jt in range(TT):
                    pt_ps = psT.tile([P, T], F32, tag="psT")
                    for it in range(TT):
                        nc.tensor.transpose(
                            pt_ps[:, it * P:(it + 1) * P],
                            p_tiles[it][:, jt * P:(jt + 1) * P],
                            ident[:],
                        )
                    PT_sb = spool.tile([P, T], F32, tag="PT")
                    nc.scalar.copy(PT_sb[:], pt_ps[:])
                    nc.tensor.matmul(
                        ao_ps[prow:prow + 64, :],
                        lhsT=_mm(V_sb[:, jt, h * 64:(h + 1) * 64], PV_MM_DT),
                        rhs=_mm(PT_sb[:], PV_MM_DT),
                        start=(jt == 0),
                        stop=(jt == TT - 1),
                    )
            nc.scalar.copy(AO_sb[:, g, :], ao_ps[:])

        # ---- output projection ----
        for tt in range(TT):
            for oh in range(2):
                ps = psA.tile([P, T], F32, tag="psA")
                for ct in range(OT):
                    nc.tensor.matmul(
                        ps[:],
                        lhsT=_mm(AO_sb[:, ct, tt * P:(tt + 1) * P], OUT_MM_DT),
                        rhs=_mm(wo_sb[:, ct, oh * 512:(oh + 1) * 512], OUT_MM_DT),
                        start=(ct == 0),
                        stop=(ct == OT - 1),
                    )
                o_sb = outp.tile([P, T], F32, tag="o")
                nc.vector.tensor_tensor(o_sb[:], ps[:], bob_sb[:, oh * 512:(oh + 1) * 512],
                                        op=add)
                nc.sync.dma_start(out_d[tt * P:(tt + 1) * P, oh * 512:(oh + 1) * 512], o_sb[:])

    nc.compile()
    if not nc.is_finalized():
        nc.finalize()
    return nc


def prep_inputs(x, Wq, bq, Wk, bk, Wv, bv, Wo, bo, head_gates, rel_bias):
    """Host-side reshapes/transposes into the layouts the device program wants."""
    x = np.asarray(x, np.float32)
    scale = 1.0 / np.sqrt(D).astype(np.float32)

    def to_kpart(w):
        # [C_in, C_out] -> [P, OT, C_out] with c_in = kt*P + p
        return np.ascontiguousarray(
            np.asarray(w, np.float32).reshape(OT, P, C).transpose(1, 0, 2))

    wq_r = to_kpart(np.asarray(Wq, np.float32).T * scale)
    wk_r = to_kpart(np.asarray(Wk, np.float32).T)
    wv_r = to_kpart(np.asarray(Wv, np.float32).T)
    wo_r = to_kpart(np.asarray(Wo, np.float32).T)

    bqp = np.ascontiguousarray((np.asarray(bq, np.float32) * scale).reshape(OT, P).T)
    bkp = np.ascontiguousarray(np.asarray(bk, np.float32).reshape(OT, P).T)
    bvb = np.ascontiguousarray(np.tile(np.asarray(bv, np.float32)[None, :], (P, 1)))
    bob = np.ascontiguousarray(np.tile(np.asarray(bo, np.float32)[None, :], (P, 1)))
    gates = np.ascontiguousarray(
        np.tile(np.asarray(head_gates, np.float32)[None, :], (P, 1)))

    idx = np.arange(T)
    rel = idx[None, :] - idx[:, None] + (MAX_POS - 1)          # [T, T]
    pb = np.asarray(rel_bias, np.float32)[rel]                 # [T, T, H]
    posb = np.ascontiguousarray(
        pb.transpose(2, 0, 1).reshape(H, TT, P, T))            # [H, TT, P, T]

    shared = dict(wq=wq_r, wk=wk_r, wv=wv_r, wo=wo_r, bqp=bqp, bkp=bkp,
                  bvb=bvb, bob=bob, gates=gates, posb=posb)

    in_maps = []
    for b in range(B):
        xT = np.ascontiguousarray(
            x[b].T.reshape(OT, P, T).transpose(1, 0, 2))       # [P, OT, T]
        in_maps.append(dict(xT=xT, **shared))
    return in_maps


_NC_CACHE = {}


def get_program():
    if "nc" not in _NC_CACHE:
        _NC_CACHE["nc"] = build_program()
    return _NC_CACHE["nc"]


def kernel(x, Wq, bq, Wk, bk, Wv, bv, Wo, bo, head_gates, rel_bias):
    nc = get_program()
    in_maps = prep_inputs(x, Wq, bq, Wk, bk, Wv, bv, Wo, bo, head_gates, rel_bias)
    res = run_bass_kernel_spmd(nc, in_maps, list(range(N_CORES)))
    return np.stack([res.results[b]["out"] for b in range(B)], axis=0)
